# revision 8
# baseline (speedup 1.0000x reference)
"""DeepGAT (3-layer GAT + BN + residual + ELU, final linear) on 8 Trainium2 cores.

Graph-parallel sharding per the problem hint: nodes (= destinations) are
sharded across the 8 cores; edges are bucketed by destination shard and
sorted by destination; source features are replicated each layer via an
AllGather of the per-shard feature table ("halo exchange" degenerates to
full replication for a random graph).

Per layer, per core:
  node phase  : h_lin = h @ W, alpha_src/dst = h_lin . att  (PE matmuls over
                the local shard, node-major via a transpose trick), written
                to a bf16 table row [h_lin(128) | a_src(8) | a_dst(8) | pad].
  AllGather   : shard tables -> full 100352-row table (bf16).
  edge phase  : per destination tile (128 dsts), per-edge rows are fetched
                with int16 dma_gather instructions (4 sub-range windows of
                25088 rows each so indices fit in int16), per-edge a_dst via
                a second local dma_gather; e = leaky_relu(a_s + a_d),
                w = exp(e)  (softmax max-subtraction is skipped: exponents
                are O(1) here, and the normalization is exact); messages
                [w*h | w] are aggregated per dst with a one-hot (is_equal
                vs iota) matmul accumulated in PSUM; out = num / den.
  BN          : per-core sums + sum-of-squares via ones-vector matmuls,
                AllReduce, then (x-mu)*g/sqrt(var+eps)+b, ELU, residual.
Final layer: h3 @ fc_w + fc_b, output assembled on host.
"""

import sys

sys.path.insert(0, "/opt/trn_rl_repo")

import numpy as np
import ml_dtypes

import concourse.bass as bass
import concourse.bacc as bacc
import concourse.mybir as mybir
import concourse.tile as tile
from concourse import library_config
from concourse.bass_utils import run_bass_kernel_spmd
from concourse.masks import make_identity

F32 = mybir.dt.float32
BF16 = mybir.dt.bfloat16
I16 = mybir.dt.int16
I32 = mybir.dt.int32
AF = mybir.ActivationFunctionType
OP = mybir.AluOpType

NCORES = 8
HID = 128
HEADS = 8
CPH = HID // HEADS
L = 3
EPS = 1e-5
SLOPE = 0.2
TCOLS = 256          # bf16 cols per gather-table row (512 B)
ROWB = 144           # used cols: h(128) + a_src(8) + a_dst(8)


def _wrap16(idx_list, pad_val=0):
    """int16 index list -> [128, n/16] wrapped-in-16, replicated x8 layout."""
    n = len(idx_list)
    assert n % 16 == 0
    out = np.full((128, n // 16), pad_val, np.int16)
    js = np.arange(n)
    row = np.asarray(idx_list, np.int64)
    assert row.max(initial=0) < 32768 and row.min(initial=0) >= 0
    for rep in range(8):
        out[16 * rep + js % 16, js // 16] = row.astype(np.int16)
    return out


def host_prep(x, edge_index, n_nodes, shard):
    """Shard + sort edges, build per-core index tensors."""
    ncores = NCORES
    ntile = (shard + 127) // 128
    nsh = ntile * 128
    nfull = nsh * ncores
    qrows = nfull // 4            # gather window size (rows), must be < 32768

    src = np.concatenate([edge_index[0], np.arange(n_nodes, dtype=np.int64)])
    dst = np.concatenate([edge_index[1], np.arange(n_nodes, dtype=np.int64)])
    owner = dst // shard
    # remap global node id -> padded table row
    srow = (src // shard) * nsh + (src % shard)

    per_core = []
    for c in range(ncores):
        sel = owner == c
        s_c = srow[sel]
        d_c = dst[sel] - c * shard           # local dst 0..shard-1
        order = np.argsort(d_c, kind="stable")
        per_core.append((s_c[order], d_c[order]))

    # per (core, tile, quarter) edge lists
    buckets = {}
    segmax = 1
    for c in range(ncores):
        s_c, d_c = per_core[c]
        t_c = d_c // 128
        q_c = s_c // qrows
        for t in range(ntile):
            tm = t_c == t
            st, dt_, qt = s_c[tm], d_c[tm], q_c[tm]
            for q in range(4):
                qm = qt == q
                buckets[(c, t, q)] = (st[qm], dt_[qm])
                segmax = max(segmax, int(qm.sum()))
    seg = ((segmax + 127) // 128) * 128
    ktot = 4 * seg // 128                    # 128-edge groups per tile

    sg16 = np.zeros((ncores, ntile, 128, 4 * seg // 16), np.int16)
    dl16 = np.zeros((ncores, ntile, 128, 4 * seg // 16), np.int16)
    dlb = np.full((ncores, ntile, 128, ktot), -1.0, np.float32)
    for c in range(ncores):
        for t in range(ntile):
            for q in range(4):
                st, dt_ = buckets[(c, t, q)]
                n = len(st)
                sl = np.zeros(seg, np.int64)
                dlq = np.zeros(seg, np.int64)
                sl[:n] = st - q * qrows
                dlq[:n] = dt_                 # local dst (for aldst gather)
                w0 = q * (seg // 16)
                sg16[c, t, :, w0:w0 + seg // 16] = _wrap16(sl)
                dl16[c, t, :, w0:w0 + seg // 16] = _wrap16(dlq)
                # dst-local-within-tile for the one-hot, -1 on padding
                g0 = q * (seg // 128)
                js = np.arange(seg)
                vals = np.full(seg, -1.0, np.float64)
                vals[:n] = (dt_ - t * 128).astype(np.float64)
                for gg in range(seg // 128):
                    blk = vals[gg * 128:(gg + 1) * 128]
                    dlb[c, t, js[gg * 128:(gg + 1) * 128] % 128, g0 + gg] = (
                        blk.astype(np.float32))
    # transposed x shards [2, 128, nsh]
    xts = []
    for c in range(ncores):
        xs = np.zeros((nsh, x.shape[1]), np.float32)
        lo, hi = c * shard, min((c + 1) * shard, n_nodes)
        xs[: hi - lo] = x[lo:hi]
        xts.append(np.ascontiguousarray(xs.T.reshape(2, 128, nsh)))
    return dict(seg=seg, ktot=ktot, ntile=ntile, nsh=nsh, nfull=nfull,
                qrows=qrows, sg16=sg16, dl16=dl16, dlb=dlb, xts=xts)


def build_program(cfg, n_nodes, shard):
    ntile, nsh, nfull, seg, ktot, qrows = (cfg["ntile"], cfg["nsh"],
                                           cfg["nfull"], cfg["seg"],
                                           cfg["ktot"], cfg["qrows"])
    kq = seg // 128                       # groups per quarter
    nc = bacc.Bacc("TRN2", target_bir_lowering=False, debug=False,
                   enable_asserts=False, num_devices=NCORES)

    xt = nc.dram_tensor("xt", [2, 128, nsh], F32, kind="ExternalInput")
    sg = nc.dram_tensor("sg", [ntile, 128, 4 * seg // 16], I16,
                        kind="ExternalInput")
    dl = nc.dram_tensor("dl", [ntile, 128, 4 * seg // 16], I16,
                        kind="ExternalInput")
    dlb = nc.dram_tensor("dlb", [ntile, 128, ktot], F32, kind="ExternalInput")
    pw = nc.dram_tensor("pw", [2, 128, HID], F32, kind="ExternalInput")
    pb = nc.dram_tensor("pb", [128, 1], F32, kind="ExternalInput")
    wext = nc.dram_tensor("wext", [L, 128, ROWB], F32, kind="ExternalInput")
    bng = nc.dram_tensor("bng", [L, 1, HID], F32, kind="ExternalInput")
    bnb = nc.dram_tensor("bnb", [L, 1, HID], F32, kind="ExternalInput")
    fcw = nc.dram_tensor("fcw", [128, 1], F32, kind="ExternalInput")
    fcb = nc.dram_tensor("fcb", [128, 1], F32, kind="ExternalInput")
    out = nc.dram_tensor("out", [nsh, 1], F32, kind="ExternalOutput")

    inv_n = 1.0 / float(n_nodes)

    with tile.TileContext(nc) as tc:
        with (tc.tile_pool(name="const", bufs=1) as cp,
              tc.tile_pool(name="sb", bufs=3) as sb,
              tc.tile_pool(name="gp", bufs=3) as gp,
              tc.tile_pool(name="pp", bufs=2, space="PSUM") as pp,
              tc.tile_pool(name="sp", bufs=1, space="PSUM") as spp,
              tc.tile_pool(name="dram", bufs=1, space="DRAM") as dp):
            nc.gpsimd.load_library(library_config.mlp)

            # ---- DRAM scratch ----
            hx_sh = dp.tile([nsh, TCOLS], F32)               # AG input
            hx_fulls = [dp.tile([nfull, TCOLS], F32, addr_space="Shared",
                                name=f"hx_full{i}") for i in range(L)]
            ad_pad = dp.tile([nsh, 128], F32)                # aldst table
            res_a = dp.tile([nsh, HID], F32)
            res_b = dp.tile([nsh, HID], F32)
            hagg_d = dp.tile([nsh, HID], F32)
            ar_in = dp.tile([2, HID], F32)
            ar_outs = [dp.tile([2, HID], F32, addr_space="Shared",
                               name=f"ar_out{i}") for i in range(L)]
            sb_row = dp.tile([2, HID], F32)                   # S,B rows bounce

            # ---- constants ----
            ident = cp.tile([128, 128], F32)
            make_identity(nc, ident[:])
            ones_col = cp.tile([128, 1], F32)
            nc.vector.memset(ones_col[:], 1.0)
            iota_i = cp.tile([128, 128], I32)
            nc.gpsimd.iota(iota_i[:], pattern=[[1, 128]], base=0,
                           channel_multiplier=0)
            iota_b = cp.tile([128, 128], F32)
            nc.vector.tensor_copy(out=iota_b[:], in_=iota_i[:])
            zrow = cp.tile([128, TCOLS], F32)
            nc.vector.memset(zrow[:], 0.0)
            pw_sb = cp.tile([128, 2 * HID], F32)
            nc.sync.dma_start(
                out=pw_sb[:].rearrange("p (k h) -> p k h", k=2),
                in_=pw.ap().rearrange("k p h -> p k h"))
            pb_sb = cp.tile([128, 1], F32)
            nc.sync.dma_start(out=pb_sb[:], in_=pb.ap()[:, :])
            wext_sb = cp.tile([128, L * ROWB], F32)
            nc.sync.dma_start(
                out=wext_sb[:].rearrange("p (l r) -> p l r", l=L),
                in_=wext.ap().rearrange("l p r -> p l r"))
            fcw_sb = cp.tile([128, 1], F32)
            nc.sync.dma_start(out=fcw_sb[:], in_=fcw.ap()[:, :])
            fcb_sb = cp.tile([128, 1], F32)
            nc.sync.dma_start(out=fcb_sb[:], in_=fcb.ap()[:, :])
            bng_sb = cp.tile([1, L * HID], F32)
            nc.sync.dma_start(out=bng_sb[:],
                              in_=bng.ap().rearrange("l o h -> o (l h)"))
            bnb_sb = cp.tile([1, L * HID], F32)
            nc.sync.dma_start(out=bnb_sb[:],
                              in_=bnb.ap().rearrange("l o h -> o (l h)"))

            # zero the pad columns of the tables once
            for t in range(ntile):
                nc.sync.dma_start(out=hx_sh[t * 128:(t + 1) * 128, :],
                                  in_=zrow[:])
                nc.sync.dma_start(out=ad_pad[t * 128:(t + 1) * 128, :],
                                  in_=zrow[:, :128])

            def elu_from_psum(ps, bias_ap):
                """ELU(ps + bias) -> returns SBUF f32 tile [128,128].

                bias_ap: per-partition [128,1] AP or None.
                """
                s0 = sb.tile([128, 128], F32, tag="elu_s0")
                if bias_ap is not None:
                    nc.scalar.activation(s0[:], ps, AF.Identity, bias=bias_ap)
                else:
                    nc.scalar.copy(s0[:], ps)
                r = sb.tile([128, 128], F32, tag="elu_r")
                nc.vector.tensor_scalar_max(out=r[:], in0=s0[:], scalar1=0.0)
                mn = sb.tile([128, 128], F32, tag="elu_mn")
                nc.vector.tensor_scalar_min(out=mn[:], in0=s0[:], scalar1=0.0)
                em = sb.tile([128, 128], F32, tag="elu_em")
                nc.scalar.activation(em[:], mn[:], AF.Exp)
                h = sb.tile([128, 128], F32, tag="elu_h")
                nc.vector.tensor_tensor(out=h[:], in0=r[:], in1=em[:],
                                        op=OP.add)
                nc.vector.tensor_scalar_add(out=h[:], in0=h[:], scalar1=-1.0)
                return h

            def write_ext(hT, li, t):
                """ext = hT.T @ wext[li] -> hx_sh + ad_pad rows of tile t."""
                ps = pp.tile([128, ROWB], F32, space="PSUM", tag="mm_ps")
                nc.tensor.matmul(ps[:], lhsT=hT[:], rhs=wext_sb[:, li * ROWB:(li + 1) * ROWB],
                                 start=True, stop=True)
                eb = sb.tile([128, ROWB], F32, tag="ext_eb")
                nc.vector.tensor_copy(out=eb[:], in_=ps[:])
                nc.sync.dma_start(
                    out=hx_sh[t * 128:(t + 1) * 128, :ROWB], in_=eb[:])
                nc.sync.dma_start(
                    out=ad_pad[t * 128:(t + 1) * 128, :8], in_=eb[:, 136:144])

            # ================= PRE: h0 = elu(x @ pw + pb) =================
            for t in range(ntile):
                ps = pp.tile([128, 128], F32, space="PSUM", tag="mm_ps")
                for k in range(2):
                    xt_sb = sb.tile([128, 128], F32, tag="xt_sb")
                    nc.sync.dma_start(
                        out=xt_sb[:],
                        in_=xt.ap()[k, :, t * 128:(t + 1) * 128])
                    nc.tensor.matmul(ps[:],
                                     lhsT=pw_sb[:, k * HID:(k + 1) * HID],
                                     rhs=xt_sb[:],
                                     start=(k == 0), stop=(k == 1))
                # ps = h0T' = (pw.T @ xT) -> [dout, node]
                h0T = elu_from_psum(ps[:], pb_sb[:, :])
                # node-major copy for residual
                tp = pp.tile([128, 128], F32, space="PSUM", tag="mm_ps")
                nc.tensor.transpose(tp[:], in_=h0T[:], identity=ident[:])
                h0 = sb.tile([128, 128], F32, tag="h0_nm")
                nc.vector.tensor_copy(out=h0[:], in_=tp[:])
                nc.sync.dma_start(out=res_a[t * 128:(t + 1) * 128, :],
                                  in_=h0[:])
                write_ext(h0T, 0, t)

            res_cur, res_nxt = res_a, res_b

            # ========================= layers =========================
            for li in range(L):
                # ---- AllGather the feature table ----
                hx_full = hx_fulls[li]
                nc.gpsimd.collective_compute(
                    "AllGather", OP.bypass,
                    replica_groups=[list(range(NCORES))],
                    ins=[hx_sh.opt()], outs=[hx_full.opt()])

                s1p = spp.tile([1, HID], F32, space="PSUM", tag="s1")
                s2p = spp.tile([1, HID], F32, space="PSUM", tag="s2")

                # ---- edge phase ----
                for t in range(ntile):
                    sg_sb = sb.tile([128, 4 * seg // 16], I16, tag="sg_sb")
                    nc.sync.dma_start(out=sg_sb[:], in_=sg.ap()[t])
                    dl_sb = sb.tile([128, 4 * seg // 16], I16, tag="dl_sb")
                    nc.sync.dma_start(out=dl_sb[:], in_=dl.ap()[t])
                    dlb_sb = sb.tile([128, ktot], F32, tag="dlb_sb")
                    nc.sync.dma_start(out=dlb_sb[:], in_=dlb.ap()[t])

                    g = gp.tile([128, ktot * TCOLS], F32, tag="g")
                    g3 = g[:].rearrange("p (k c) -> p k c", c=TCOLS)
                    for q in range(4):
                        nc.gpsimd.dma_gather(
                            out_ap=g3[:, q * kq:(q + 1) * kq, :],
                            in_ap=hx_full[q * qrows:(q + 1) * qrows, :],
                            idxs_ap=sg_sb[:, q * (seg // 16):(q + 1) * (seg // 16)],
                            num_idxs=seg, num_idxs_reg=seg,
                            elem_size=TCOLS, single_packet=False)
                    ad = gp.tile([128, ktot * 128], F32, tag="ad")
                    ad3 = ad[:].rearrange("p (k c) -> p k c", c=128)
                    for q in range(4):
                        nc.gpsimd.dma_gather(
                            out_ap=ad3[:, q * kq:(q + 1) * kq, :],
                            in_ap=ad_pad[:, :],
                            idxs_ap=dl_sb[:, q * (seg // 16):(q + 1) * (seg // 16)],
                            num_idxs=seg, num_idxs_reg=seg,
                            elem_size=128, single_packet=False)

                    # one-hot selection matrices, all groups at once
                    mt = gp.tile([128, ktot * 128], F32, tag="mt")
                    nc.vector.tensor_tensor(
                        out=mt[:].rearrange("p (k c) -> p k c", c=128),
                        in0=dlb_sb[:].unsqueeze(-1).to_broadcast(
                            [128, ktot, 128]),
                        in1=iota_b[:].unsqueeze(1).to_broadcast(
                            [128, ktot, 128]),
                        op=OP.is_equal)

                    # e = a_src + a_dst ; lrelu; w = exp -> msg cols
                    e_sb = sb.tile([128, ktot * 8], F32, tag="e_sb")
                    e3 = e_sb[:].rearrange("p (k c) -> p k c", c=8)
                    nc.vector.tensor_tensor(
                        out=e3, in0=g3[:, :, 128:136], in1=ad3[:, :, 0:8],
                        op=OP.add)
                    er = sb.tile([128, ktot * 8], F32, tag="er")
                    nc.vector.tensor_scalar_max(out=er[:], in0=e_sb[:],
                                                scalar1=0.0)
                    nc.vector.tensor_scalar_min(out=e_sb[:], in0=e_sb[:],
                                                scalar1=0.0)
                    nc.vector.tensor_scalar_mul(out=e_sb[:], in0=e_sb[:],
                                                scalar1=SLOPE)
                    nc.vector.tensor_tensor(out=e_sb[:], in0=e_sb[:],
                                            in1=er[:], op=OP.add)
                    nc.scalar.activation(g3[:, :, 128:136], e3, AF.Exp)
                    # msg h-cols *= w (broadcast 16x per head)
                    nc.vector.tensor_tensor(
                        out=g3[:, :, 0:128].rearrange("p k (a b) -> p k a b",
                                                      a=8),
                        in0=g3[:, :, 0:128].rearrange("p k (a b) -> p k a b",
                                                      a=8),
                        in1=g3[:, :, 128:136].unsqueeze(-1).to_broadcast(
                            [128, ktot, 8, 16]),
                        op=OP.mult)

                    agg = pp.tile([128, 136], F32, space="PSUM", tag="agg")
                    for gg in range(ktot):
                        nc.tensor.matmul(
                            agg[:], lhsT=mt[:, gg * 128:(gg + 1) * 128],
                            rhs=g3[:, gg, 0:136],
                            start=(gg == 0), stop=(gg == ktot - 1))

                    den = sb.tile([128, 8], F32, tag="den")
                    nc.vector.tensor_scalar_add(out=den[:],
                                                in0=agg[:, 128:136],
                                                scalar1=1e-16)
                    nc.vector.reciprocal(out=den[:], in_=den[:])
                    hag = sb.tile([128, 128], F32, tag="hag")
                    nc.vector.tensor_tensor(
                        out=hag[:].rearrange("p (a b) -> p a b", a=8),
                        in0=agg[:, 0:128].rearrange("p (a b) -> p a b", a=8),
                        in1=den[:].unsqueeze(-1).to_broadcast([128, 8, 16]),
                        op=OP.mult)
                    nc.sync.dma_start(out=hagg_d[t * 128:(t + 1) * 128, :],
                                      in_=hag[:])
                    sq = sb.tile([128, 128], F32, tag="sq")
                    nc.scalar.activation(sq[:], hag[:], AF.Square)
                    nc.tensor.matmul(s1p[:], lhsT=ones_col[:], rhs=hag[:],
                                     start=(t == 0), stop=(t == ntile - 1))
                    nc.tensor.matmul(s2p[:], lhsT=ones_col[:], rhs=sq[:],
                                     start=(t == 0), stop=(t == ntile - 1))

                # ---- BN stats AllReduce ----
                st1 = sb.tile([1, HID], F32, tag="st1")
                nc.vector.tensor_copy(out=st1[:], in_=s1p[:])
                st2s = sb.tile([1, HID], F32, tag="st2s")
                nc.vector.tensor_copy(out=st2s[:], in_=s2p[:])
                nc.sync.dma_start(out=ar_in[0:1, :], in_=st1[:])
                nc.sync.dma_start(out=ar_in[1:2, :], in_=st2s[:])
                ar_out = ar_outs[li]
                nc.gpsimd.collective_compute(
                    "AllReduce", OP.add,
                    replica_groups=[list(range(NCORES))],
                    ins=[ar_in.opt()], outs=[ar_out.opt()])
                g1 = sb.tile([1, HID], F32, tag="g1r")
                nc.sync.dma_start(out=g1[:], in_=ar_out[0:1, :])
                g2 = sb.tile([1, HID], F32, tag="g2r")
                nc.sync.dma_start(out=g2[:], in_=ar_out[1:2, :])
                # mu, var, S = g/sqrt(var+eps), B = b - mu*S  (partition 0/1)
                mu = sb.tile([1, HID], F32, tag="mu")
                nc.vector.tensor_scalar_mul(out=mu[:], in0=g1[:],
                                            scalar1=inv_n)
                var = sb.tile([1, HID], F32, tag="var")
                nc.vector.tensor_scalar_mul(out=var[:], in0=g2[:],
                                            scalar1=inv_n)
                musq = sb.tile([1, HID], F32, tag="musq")
                nc.vector.tensor_tensor(out=musq[:], in0=mu[:], in1=mu[:],
                                        op=OP.mult)
                nc.vector.tensor_tensor(out=var[:], in0=var[:], in1=musq[:],
                                        op=OP.subtract)
                nc.vector.tensor_scalar_add(out=var[:], in0=var[:],
                                            scalar1=EPS)
                sd = sb.tile([1, HID], F32, tag="sd")
                nc.scalar.activation(sd[:], var[:], AF.Sqrt)
                nc.vector.reciprocal(out=sd[:], in_=sd[:])
                srow = sb.tile([1, HID], F32, tag="srow")
                nc.vector.tensor_tensor(out=srow[:], in0=sd[:],
                                        in1=bng_sb[0:1, li * HID:(li + 1) * HID], op=OP.mult)
                brow = sb.tile([1, HID], F32, tag="brow")
                nc.vector.tensor_tensor(out=brow[:], in0=mu[:], in1=srow[:],
                                        op=OP.mult)
                nc.vector.tensor_tensor(out=brow[:],
                                        in0=bnb_sb[0:1, li * HID:(li + 1) * HID],
                                        in1=brow[:], op=OP.subtract)
                nc.sync.dma_start(out=sb_row[0:1, :], in_=srow[:])
                nc.sync.dma_start(out=sb_row[1:2, :], in_=brow[:])
                s_bc = sb.tile([128, HID], F32, tag="s_bc")
                nc.sync.dma_start(out=s_bc[:],
                                  in_=sb_row[0:1, :].partition_broadcast(
                                      128).squeeze(1))
                b_bc = sb.tile([128, HID], F32, tag="b_bc")
                nc.sync.dma_start(out=b_bc[:],
                                  in_=sb_row[1:2, :].partition_broadcast(
                                      128).squeeze(1))

                # ---- node phase: BN + elu + residual (+ next tables) ----
                for t in range(ntile):
                    hg = sb.tile([128, 128], F32, tag="hg_n")
                    nc.sync.dma_start(out=hg[:],
                                      in_=hagg_d[t * 128:(t + 1) * 128, :])
                    rs = sb.tile([128, 128], F32, tag="rs_n")
                    nc.sync.dma_start(out=rs[:],
                                      in_=res_cur[t * 128:(t + 1) * 128, :])
                    u = sb.tile([128, 128], F32, tag="u_n")
                    nc.vector.tensor_tensor(out=u[:], in0=hg[:], in1=s_bc[:],
                                            op=OP.mult)
                    nc.vector.tensor_tensor(out=u[:], in0=u[:], in1=b_bc[:],
                                            op=OP.add)
                    r = sb.tile([128, 128], F32, tag="r_n")
                    nc.vector.tensor_scalar_max(out=r[:], in0=u[:],
                                                scalar1=0.0)
                    mn = sb.tile([128, 128], F32, tag="mn_n")
                    nc.vector.tensor_scalar_min(out=mn[:], in0=u[:],
                                                scalar1=0.0)
                    em = sb.tile([128, 128], F32, tag="em_n")
                    nc.scalar.activation(em[:], mn[:], AF.Exp)
                    hn = sb.tile([128, 128], F32, tag="hn_n")
                    nc.vector.tensor_tensor(out=hn[:], in0=r[:], in1=em[:],
                                            op=OP.add)
                    nc.vector.tensor_scalar_add(out=hn[:], in0=hn[:],
                                                scalar1=-1.0)
                    nc.vector.tensor_tensor(out=hn[:], in0=hn[:], in1=rs[:],
                                            op=OP.add)
                    if li < L - 1:
                        nc.sync.dma_start(
                            out=res_nxt[t * 128:(t + 1) * 128, :], in_=hn[:])
                    tp = pp.tile([128, 128], F32, space="PSUM", tag="mm_ps")
                    nc.tensor.transpose(tp[:], in_=hn[:], identity=ident[:])
                    hT = sb.tile([128, 128], F32, tag="hT_n")
                    nc.vector.tensor_copy(out=hT[:], in_=tp[:])
                    if li < L - 1:
                        write_ext(hT, li + 1, t)
                    else:
                        fps = pp.tile([128, 1], F32, space="PSUM",
                                      tag="mm_ps")
                        nc.tensor.matmul(fps[:], lhsT=hT[:], rhs=fcw_sb[:],
                                         start=True, stop=True)
                        ov = sb.tile([128, 1], F32, tag="ov")
                        nc.scalar.activation(ov[:], fps[:], AF.Identity,
                                             bias=fcb_sb[:, 0:1])
                        nc.sync.dma_start(
                            out=out.ap()[t * 128:(t + 1) * 128, :],
                            in_=ov[:])
                res_cur, res_nxt = res_nxt, res_cur

    nc.compile()
    return nc


def _run(x, edge_index, proj_w, proj_b, W, att_src, att_dst, conv_b,
         bn_g, bn_b, fc_w, fc_b, n_nodes, shard, trace=False):
    x = np.asarray(x, np.float32)
    edge_index = np.asarray(edge_index, np.int64)
    cfg = host_prep(x, edge_index, n_nodes, shard)
    nc = build_program(cfg, n_nodes, shard)

    # fold attention vectors into the layer weight: B = W @ A, A[(h,c),h']=a
    wext_np = np.zeros((L, 128, ROWB), np.float32)
    for li in range(L):
        A_s = np.zeros((HID, HEADS), np.float32)
        A_d = np.zeros((HID, HEADS), np.float32)
        for h in range(HEADS):
            A_s[h * CPH:(h + 1) * CPH, h] = att_src[li, h]
            A_d[h * CPH:(h + 1) * CPH, h] = att_dst[li, h]
        wext_np[li, :, :128] = W[li]
        wext_np[li, :, 128:136] = W[li] @ A_s
        wext_np[li, :, 136:144] = W[li] @ A_d

    common = {
        "pw": np.ascontiguousarray(
            np.asarray(proj_w, np.float32).reshape(2, 128, HID)),
        "pb": np.asarray(proj_b, np.float32).reshape(128, 1),
        "wext": wext_np,
        "bng": np.asarray(bn_g, np.float32).reshape(L, 1, HID),
        "bnb": np.asarray(bn_b, np.float32).reshape(L, 1, HID),
        "fcw": np.asarray(fc_w, np.float32).reshape(128, 1),
        "fcb": np.full((128, 1), np.asarray(fc_b).reshape(-1)[0], np.float32),
    }
    in_maps = []
    for c in range(NCORES):
        m = dict(common)
        m["xt"] = cfg["xts"][c]
        m["sg"] = cfg["sg16"][c]
        m["dl"] = cfg["dl16"][c]
        m["dlb"] = cfg["dlb"][c]
        in_maps.append(m)

    res = run_bass_kernel_spmd(nc, in_maps, core_ids=list(range(NCORES)),
                               trace=trace)
    outs = [res.results[c]["out"][:min(shard, n_nodes - c * shard)]
            for c in range(NCORES)]
    full = np.concatenate(outs, axis=0).astype(np.float32)
    return full, res


def kernel(x, edge_index, proj_w, proj_b, W, att_src, att_dst, conv_b,
           bn_g, bn_b, fc_w, fc_b):
    full, _ = _run(x, edge_index, proj_w, proj_b, W, att_src, att_dst,
                   conv_b, bn_g, bn_b, fc_w, fc_b,
                   n_nodes=100000, shard=12500)
    return full


# revision 9
# speedup vs baseline: 1.6358x; 1.6358x over previous
"""DeepGAT (3-layer GAT + BN + residual + ELU, final linear) on 8 Trainium2 cores.

Graph-parallel sharding per the problem hint: nodes (= destinations) are
sharded across the 8 cores; edges are bucketed by destination shard and
sorted by destination; source features are replicated each layer via an
AllGather of the per-shard feature table ("halo exchange" degenerates to
full replication for a random graph).

Per layer, per core:
  node phase  : h_lin = h @ W, alpha_src/dst = h_lin . att  (PE matmuls over
                the local shard, node-major via a transpose trick), written
                to a bf16 table row [h_lin(128) | a_src(8) | a_dst(8) | pad].
  AllGather   : shard tables -> full 100352-row table (bf16).
  edge phase  : per destination tile (128 dsts), per-edge rows are fetched
                with int16 dma_gather instructions (4 sub-range windows of
                25088 rows each so indices fit in int16), per-edge a_dst via
                a second local dma_gather; e = leaky_relu(a_s + a_d),
                w = exp(e)  (softmax max-subtraction is skipped: exponents
                are O(1) here, and the normalization is exact); messages
                [w*h | w] are aggregated per dst with a one-hot (is_equal
                vs iota) matmul accumulated in PSUM; out = num / den.
  BN          : per-core sums + sum-of-squares via ones-vector matmuls,
                AllReduce, then (x-mu)*g/sqrt(var+eps)+b, ELU, residual.
Final layer: h3 @ fc_w + fc_b, output assembled on host.
"""

import sys

sys.path.insert(0, "/opt/trn_rl_repo")

import numpy as np
import ml_dtypes

import concourse.bass as bass
import concourse.bacc as bacc
import concourse.mybir as mybir
import concourse.tile as tile
from concourse import library_config
from concourse.bass_utils import run_bass_kernel_spmd
from concourse.masks import make_identity

F32 = mybir.dt.float32
BF16 = mybir.dt.bfloat16
I16 = mybir.dt.int16
I32 = mybir.dt.int32
AF = mybir.ActivationFunctionType
OP = mybir.AluOpType

NCORES = 8
HID = 128
HEADS = 8
CPH = HID // HEADS
L = 3
EPS = 1e-5
SLOPE = 0.2
TCOLS = 256          # bf16 cols per gather-table row (512 B)
ROWB = 144           # used cols: h(128) + a_src(8) + a_dst(8)


def _wrap16(idx_list, pad_val=0):
    """int16 index list -> [128, n/16] wrapped-in-16, replicated x8 layout."""
    n = len(idx_list)
    assert n % 16 == 0
    out = np.full((128, n // 16), pad_val, np.int16)
    js = np.arange(n)
    row = np.asarray(idx_list, np.int64)
    assert row.max(initial=0) < 32768 and row.min(initial=0) >= 0
    for rep in range(8):
        out[16 * rep + js % 16, js // 16] = row.astype(np.int16)
    return out


def host_prep(x, edge_index, n_nodes, shard):
    """Shard + sort edges, build per-core index tensors."""
    ncores = NCORES
    ntile = (shard + 127) // 128
    nsh = ntile * 128
    nfull = nsh * ncores
    qrows = nfull // 4            # gather window size (rows), must be < 32768

    src = np.concatenate([edge_index[0], np.arange(n_nodes, dtype=np.int64)])
    dst = np.concatenate([edge_index[1], np.arange(n_nodes, dtype=np.int64)])
    owner = dst // shard
    # remap global node id -> padded table row
    srow = (src // shard) * nsh + (src % shard)

    per_core = []
    for c in range(ncores):
        sel = owner == c
        s_c = srow[sel]
        d_c = dst[sel] - c * shard           # local dst 0..shard-1
        order = np.argsort(d_c, kind="stable")
        per_core.append((s_c[order], d_c[order]))

    # per (core, tile, quarter) edge lists
    buckets = {}
    segmax = 1
    for c in range(ncores):
        s_c, d_c = per_core[c]
        t_c = d_c // 128
        q_c = s_c // qrows
        for t in range(ntile):
            tm = t_c == t
            st, dt_, qt = s_c[tm], d_c[tm], q_c[tm]
            for q in range(4):
                qm = qt == q
                buckets[(c, t, q)] = (st[qm], dt_[qm])
                segmax = max(segmax, int(qm.sum()))
    seg = ((segmax + 127) // 128) * 128
    ktot = 4 * seg // 128                    # 128-edge groups per tile

    sg16 = np.zeros((ncores, ntile, 128, 4 * seg // 16), np.int16)
    dl16 = np.zeros((ncores, ntile, 128, 4 * seg // 16), np.int16)
    dlb = np.full((ncores, ntile, 128, ktot), -1.0, np.float32)
    for c in range(ncores):
        for t in range(ntile):
            for q in range(4):
                st, dt_ = buckets[(c, t, q)]
                n = len(st)
                sl = np.zeros(seg, np.int64)
                dlq = np.zeros(seg, np.int64)
                sl[:n] = st - q * qrows
                dlq[:n] = dt_                 # local dst (for aldst gather)
                w0 = q * (seg // 16)
                sg16[c, t, :, w0:w0 + seg // 16] = _wrap16(sl)
                dl16[c, t, :, w0:w0 + seg // 16] = _wrap16(dlq)
                # dst-local-within-tile for the one-hot, -1 on padding
                g0 = q * (seg // 128)
                js = np.arange(seg)
                vals = np.full(seg, -1.0, np.float64)
                vals[:n] = (dt_ - t * 128).astype(np.float64)
                for gg in range(seg // 128):
                    blk = vals[gg * 128:(gg + 1) * 128]
                    dlb[c, t, js[gg * 128:(gg + 1) * 128] % 128, g0 + gg] = (
                        blk.astype(np.float32))
    # transposed x shards [2, 128, nsh]
    xts = []
    for c in range(ncores):
        xs = np.zeros((nsh, x.shape[1]), np.float32)
        lo, hi = c * shard, min((c + 1) * shard, n_nodes)
        xs[: hi - lo] = x[lo:hi]
        xts.append(np.ascontiguousarray(xs.T.reshape(2, 128, nsh)))
    return dict(seg=seg, ktot=ktot, ntile=ntile, nsh=nsh, nfull=nfull,
                qrows=qrows, sg16=sg16, dl16=dl16, dlb=dlb, xts=xts)


def build_program(cfg, n_nodes, shard):
    ntile, nsh, nfull, seg, ktot, qrows = (cfg["ntile"], cfg["nsh"],
                                           cfg["nfull"], cfg["seg"],
                                           cfg["ktot"], cfg["qrows"])
    kq = seg // 128                       # groups per quarter
    nc = bacc.Bacc("TRN2", target_bir_lowering=False, debug=False,
                   enable_asserts=False, num_devices=NCORES)

    xt = nc.dram_tensor("xt", [2, 128, nsh], F32, kind="ExternalInput")
    sg = nc.dram_tensor("sg", [ntile, 128, 4 * seg // 16], I16,
                        kind="ExternalInput")
    dl = nc.dram_tensor("dl", [ntile, 128, 4 * seg // 16], I16,
                        kind="ExternalInput")
    dlb = nc.dram_tensor("dlb", [ntile, 128, ktot], F32, kind="ExternalInput")
    pw = nc.dram_tensor("pw", [2, 128, HID], F32, kind="ExternalInput")
    pb = nc.dram_tensor("pb", [128, 1], F32, kind="ExternalInput")
    wext = nc.dram_tensor("wext", [L, 128, ROWB], F32, kind="ExternalInput")
    bng = nc.dram_tensor("bng", [L, 1, HID], F32, kind="ExternalInput")
    bnb = nc.dram_tensor("bnb", [L, 1, HID], F32, kind="ExternalInput")
    fcw = nc.dram_tensor("fcw", [128, 1], F32, kind="ExternalInput")
    fcb = nc.dram_tensor("fcb", [128, 1], F32, kind="ExternalInput")
    out = nc.dram_tensor("out", [nsh, 1], F32, kind="ExternalOutput")

    inv_n = 1.0 / float(n_nodes)

    with tile.TileContext(nc) as tc:
        with (tc.tile_pool(name="const", bufs=1) as cp,
              tc.tile_pool(name="sb", bufs=3) as sb,
              tc.tile_pool(name="gp", bufs=3) as gp,
              tc.tile_pool(name="pp", bufs=2, space="PSUM") as pp,
              tc.tile_pool(name="sp", bufs=1, space="PSUM") as spp,
              tc.tile_pool(name="dram", bufs=1, space="DRAM") as dp):
            nc.gpsimd.load_library(library_config.mlp)

            # ---- DRAM scratch ----
            hx_sh = dp.tile([nsh, TCOLS], F32)               # AG input
            hx_fulls = [dp.tile([nfull, TCOLS], F32, addr_space="Shared",
                                name=f"hx_full{i}") for i in range(L)]
            ad_pad = dp.tile([nsh, 128], F32)                # aldst table
            res_a = dp.tile([nsh, HID], F32)
            res_b = dp.tile([nsh, HID], F32)
            hagg_d = dp.tile([nsh, HID], F32)
            ar_in = dp.tile([2, HID], F32)
            ar_outs = [dp.tile([2, HID], F32, addr_space="Shared",
                               name=f"ar_out{i}") for i in range(L)]
            sb_row = dp.tile([2, HID], F32)                   # S,B rows bounce

            # ---- constants ----
            ident = cp.tile([128, 128], F32)
            make_identity(nc, ident[:])
            ones_col = cp.tile([128, 1], F32)
            nc.vector.memset(ones_col[:], 1.0)
            iota_i = cp.tile([128, 128], I32)
            nc.gpsimd.iota(iota_i[:], pattern=[[1, 128]], base=0,
                           channel_multiplier=0)
            iota_b = cp.tile([128, 128], F32)
            nc.vector.tensor_copy(out=iota_b[:], in_=iota_i[:])
            zrow = cp.tile([128, TCOLS], F32)
            nc.vector.memset(zrow[:], 0.0)
            pw_sb = cp.tile([128, 2 * HID], F32)
            nc.sync.dma_start(
                out=pw_sb[:].rearrange("p (k h) -> p k h", k=2),
                in_=pw.ap().rearrange("k p h -> p k h"))
            pb_sb = cp.tile([128, 1], F32)
            nc.sync.dma_start(out=pb_sb[:], in_=pb.ap()[:, :])
            wext_sb = cp.tile([128, L * ROWB], F32)
            nc.sync.dma_start(
                out=wext_sb[:].rearrange("p (l r) -> p l r", l=L),
                in_=wext.ap().rearrange("l p r -> p l r"))
            fcw_sb = cp.tile([128, 1], F32)
            nc.sync.dma_start(out=fcw_sb[:], in_=fcw.ap()[:, :])
            fcb_sb = cp.tile([128, 1], F32)
            nc.sync.dma_start(out=fcb_sb[:], in_=fcb.ap()[:, :])
            bng_sb = cp.tile([1, L * HID], F32)
            nc.sync.dma_start(out=bng_sb[:],
                              in_=bng.ap().rearrange("l o h -> o (l h)"))
            bnb_sb = cp.tile([1, L * HID], F32)
            nc.sync.dma_start(out=bnb_sb[:],
                              in_=bnb.ap().rearrange("l o h -> o (l h)"))

            # zero the pad columns of the tables once
            for t in range(ntile):
                nc.sync.dma_start(out=hx_sh[t * 128:(t + 1) * 128, :],
                                  in_=zrow[:])
                nc.sync.dma_start(out=ad_pad[t * 128:(t + 1) * 128, :],
                                  in_=zrow[:, :128])

            def elu_from_psum(ps, bias_ap):
                """ELU(ps + bias) -> returns SBUF f32 tile [128,128].

                bias_ap: per-partition [128,1] AP or None.
                """
                s0 = sb.tile([128, 128], F32, tag="elu_s0")
                if bias_ap is not None:
                    nc.scalar.activation(s0[:], ps, AF.Identity, bias=bias_ap)
                else:
                    nc.scalar.copy(s0[:], ps)
                r = sb.tile([128, 128], F32, tag="elu_r")
                nc.vector.tensor_scalar_max(out=r[:], in0=s0[:], scalar1=0.0)
                mn = sb.tile([128, 128], F32, tag="elu_mn")
                nc.vector.tensor_scalar_min(out=mn[:], in0=s0[:], scalar1=0.0)
                em = sb.tile([128, 128], F32, tag="elu_em")
                nc.scalar.activation(em[:], mn[:], AF.Exp)
                h = sb.tile([128, 128], F32, tag="elu_h")
                nc.vector.tensor_tensor(out=h[:], in0=r[:], in1=em[:],
                                        op=OP.add)
                nc.vector.tensor_scalar_add(out=h[:], in0=h[:], scalar1=-1.0)
                return h

            def write_ext(hT, li, t):
                """ext = hT.T @ wext[li] -> hx_sh + ad_pad rows of tile t."""
                ps = pp.tile([128, ROWB], F32, space="PSUM", tag="mm_ps")
                nc.tensor.matmul(ps[:], lhsT=hT[:], rhs=wext_sb[:, li * ROWB:(li + 1) * ROWB],
                                 start=True, stop=True)
                eb = sb.tile([128, ROWB], F32, tag="ext_eb")
                nc.vector.tensor_copy(out=eb[:], in_=ps[:])
                nc.sync.dma_start(
                    out=hx_sh[t * 128:(t + 1) * 128, :ROWB], in_=eb[:])
                nc.sync.dma_start(
                    out=ad_pad[t * 128:(t + 1) * 128, :8], in_=eb[:, 136:144])

            # ================= PRE: h0 = elu(x @ pw + pb) =================
            for t in range(ntile):
                ps = pp.tile([128, 128], F32, space="PSUM", tag="mm_ps")
                for k in range(2):
                    xt_sb = sb.tile([128, 128], F32, tag="xt_sb")
                    nc.sync.dma_start(
                        out=xt_sb[:],
                        in_=xt.ap()[k, :, t * 128:(t + 1) * 128])
                    nc.tensor.matmul(ps[:],
                                     lhsT=pw_sb[:, k * HID:(k + 1) * HID],
                                     rhs=xt_sb[:],
                                     start=(k == 0), stop=(k == 1))
                # ps = h0T' = (pw.T @ xT) -> [dout, node]
                h0T = elu_from_psum(ps[:], pb_sb[:, :])
                # node-major copy for residual
                tp = pp.tile([128, 128], F32, space="PSUM", tag="mm_ps")
                nc.tensor.transpose(tp[:], in_=h0T[:], identity=ident[:])
                h0 = sb.tile([128, 128], F32, tag="h0_nm")
                nc.vector.tensor_copy(out=h0[:], in_=tp[:])
                nc.sync.dma_start(out=res_a[t * 128:(t + 1) * 128, :],
                                  in_=h0[:])
                write_ext(h0T, 0, t)

            res_cur, res_nxt = res_a, res_b

            # ========================= layers =========================
            for li in range(L):
                # ---- AllGather the feature table ----
                hx_full = hx_fulls[li]
                nc.gpsimd.collective_compute(
                    "AllGather", OP.bypass,
                    replica_groups=[list(range(NCORES))],
                    ins=[hx_sh.opt()], outs=[hx_full.opt()])

                s1p = spp.tile([1, HID], F32, space="PSUM", tag="s1")
                s2p = spp.tile([1, HID], F32, space="PSUM", tag="s2")

                # ---- edge phase ----
                for t in range(ntile):
                    sg_sb = sb.tile([128, 4 * seg // 16], I16, tag="sg_sb")
                    nc.sync.dma_start(out=sg_sb[:], in_=sg.ap()[t])
                    dlb_sb = sb.tile([128, ktot], F32, tag="dlb_sb")
                    nc.sync.dma_start(out=dlb_sb[:], in_=dlb.ap()[t])

                    g = gp.tile([128, ktot * TCOLS], F32, tag="g")
                    g3 = g[:].rearrange("p (k c) -> p k c", c=TCOLS)
                    for q in range(4):
                        nc.gpsimd.dma_gather(
                            out_ap=g3[:, q * kq:(q + 1) * kq, :],
                            in_ap=hx_full[q * qrows:(q + 1) * qrows, :],
                            idxs_ap=sg_sb[:, q * (seg // 16):(q + 1) * (seg // 16)],
                            num_idxs=seg, num_idxs_reg=seg,
                            elem_size=TCOLS, single_packet=False)
                    # one-hot selection matrices, all groups at once
                    mt = gp.tile([128, ktot * 128], F32, tag="mt")
                    nc.vector.tensor_tensor(
                        out=mt[:].rearrange("p (k c) -> p k c", c=128),
                        in0=dlb_sb[:].unsqueeze(-1).to_broadcast(
                            [128, ktot, 128]),
                        in1=iota_b[:].unsqueeze(1).to_broadcast(
                            [128, ktot, 128]),
                        op=OP.is_equal)

                    # per-edge a_dst: Mt^T (PE transpose) @ aldst_tile
                    adt = sb.tile([128, 8], F32, tag="adt")
                    nc.sync.dma_start(
                        out=adt[:],
                        in_=ad_pad[t * 128:(t + 1) * 128, 0:8])
                    adall = sb.tile([128, ktot * 8], F32, tag="adall")
                    ad3 = adall[:].rearrange("p (k c) -> p k c", c=8)
                    for gg in range(ktot):
                        tpo = pp.tile([128, 128], F32, space="PSUM",
                                      tag="mm_ps")
                        nc.tensor.transpose(
                            tpo[:], in_=mt[:, gg * 128:(gg + 1) * 128],
                            identity=ident[:])
                        osb = sb.tile([128, 128], F32, tag="osb")
                        nc.vector.tensor_copy(out=osb[:], in_=tpo[:])
                        adp = pp.tile([128, 8], F32, space="PSUM",
                                      tag="ad_ps")
                        nc.tensor.matmul(adp[:], lhsT=osb[:], rhs=adt[:],
                                         start=True, stop=True)
                        nc.vector.tensor_copy(out=ad3[:, gg, :], in_=adp[:])

                    # e = a_src + a_dst ; lrelu; w = exp -> msg cols
                    e_sb = sb.tile([128, ktot * 8], F32, tag="e_sb")
                    e3 = e_sb[:].rearrange("p (k c) -> p k c", c=8)
                    nc.vector.tensor_tensor(
                        out=e3, in0=g3[:, :, 128:136], in1=ad3[:, :, 0:8],
                        op=OP.add)
                    er = sb.tile([128, ktot * 8], F32, tag="er")
                    nc.vector.tensor_scalar_max(out=er[:], in0=e_sb[:],
                                                scalar1=0.0)
                    nc.vector.tensor_scalar_min(out=e_sb[:], in0=e_sb[:],
                                                scalar1=0.0)
                    nc.vector.tensor_scalar_mul(out=e_sb[:], in0=e_sb[:],
                                                scalar1=SLOPE)
                    nc.vector.tensor_tensor(out=e_sb[:], in0=e_sb[:],
                                            in1=er[:], op=OP.add)
                    nc.scalar.activation(g3[:, :, 128:136], e3, AF.Exp)
                    # msg h-cols *= w (broadcast 16x per head)
                    nc.vector.tensor_tensor(
                        out=g3[:, :, 0:128].rearrange("p k (a b) -> p k a b",
                                                      a=8),
                        in0=g3[:, :, 0:128].rearrange("p k (a b) -> p k a b",
                                                      a=8),
                        in1=g3[:, :, 128:136].unsqueeze(-1).to_broadcast(
                            [128, ktot, 8, 16]),
                        op=OP.mult)

                    agg = pp.tile([128, 136], F32, space="PSUM", tag="agg")
                    for gg in range(ktot):
                        nc.tensor.matmul(
                            agg[:], lhsT=mt[:, gg * 128:(gg + 1) * 128],
                            rhs=g3[:, gg, 0:136],
                            start=(gg == 0), stop=(gg == ktot - 1))

                    den = sb.tile([128, 8], F32, tag="den")
                    nc.vector.tensor_scalar_add(out=den[:],
                                                in0=agg[:, 128:136],
                                                scalar1=1e-16)
                    nc.vector.reciprocal(out=den[:], in_=den[:])
                    hag = sb.tile([128, 128], F32, tag="hag")
                    nc.vector.tensor_tensor(
                        out=hag[:].rearrange("p (a b) -> p a b", a=8),
                        in0=agg[:, 0:128].rearrange("p (a b) -> p a b", a=8),
                        in1=den[:].unsqueeze(-1).to_broadcast([128, 8, 16]),
                        op=OP.mult)
                    nc.sync.dma_start(out=hagg_d[t * 128:(t + 1) * 128, :],
                                      in_=hag[:])
                    sq = sb.tile([128, 128], F32, tag="sq")
                    nc.scalar.activation(sq[:], hag[:], AF.Square)
                    nc.tensor.matmul(s1p[:], lhsT=ones_col[:], rhs=hag[:],
                                     start=(t == 0), stop=(t == ntile - 1))
                    nc.tensor.matmul(s2p[:], lhsT=ones_col[:], rhs=sq[:],
                                     start=(t == 0), stop=(t == ntile - 1))

                # ---- BN stats AllReduce ----
                st1 = sb.tile([1, HID], F32, tag="st1")
                nc.vector.tensor_copy(out=st1[:], in_=s1p[:])
                st2s = sb.tile([1, HID], F32, tag="st2s")
                nc.vector.tensor_copy(out=st2s[:], in_=s2p[:])
                nc.sync.dma_start(out=ar_in[0:1, :], in_=st1[:])
                nc.sync.dma_start(out=ar_in[1:2, :], in_=st2s[:])
                ar_out = ar_outs[li]
                nc.gpsimd.collective_compute(
                    "AllReduce", OP.add,
                    replica_groups=[list(range(NCORES))],
                    ins=[ar_in.opt()], outs=[ar_out.opt()])
                g1 = sb.tile([1, HID], F32, tag="g1r")
                nc.sync.dma_start(out=g1[:], in_=ar_out[0:1, :])
                g2 = sb.tile([1, HID], F32, tag="g2r")
                nc.sync.dma_start(out=g2[:], in_=ar_out[1:2, :])
                # mu, var, S = g/sqrt(var+eps), B = b - mu*S  (partition 0/1)
                mu = sb.tile([1, HID], F32, tag="mu")
                nc.vector.tensor_scalar_mul(out=mu[:], in0=g1[:],
                                            scalar1=inv_n)
                var = sb.tile([1, HID], F32, tag="var")
                nc.vector.tensor_scalar_mul(out=var[:], in0=g2[:],
                                            scalar1=inv_n)
                musq = sb.tile([1, HID], F32, tag="musq")
                nc.vector.tensor_tensor(out=musq[:], in0=mu[:], in1=mu[:],
                                        op=OP.mult)
                nc.vector.tensor_tensor(out=var[:], in0=var[:], in1=musq[:],
                                        op=OP.subtract)
                nc.vector.tensor_scalar_add(out=var[:], in0=var[:],
                                            scalar1=EPS)
                sd = sb.tile([1, HID], F32, tag="sd")
                nc.scalar.activation(sd[:], var[:], AF.Sqrt)
                nc.vector.reciprocal(out=sd[:], in_=sd[:])
                srow = sb.tile([1, HID], F32, tag="srow")
                nc.vector.tensor_tensor(out=srow[:], in0=sd[:],
                                        in1=bng_sb[0:1, li * HID:(li + 1) * HID], op=OP.mult)
                brow = sb.tile([1, HID], F32, tag="brow")
                nc.vector.tensor_tensor(out=brow[:], in0=mu[:], in1=srow[:],
                                        op=OP.mult)
                nc.vector.tensor_tensor(out=brow[:],
                                        in0=bnb_sb[0:1, li * HID:(li + 1) * HID],
                                        in1=brow[:], op=OP.subtract)
                nc.sync.dma_start(out=sb_row[0:1, :], in_=srow[:])
                nc.sync.dma_start(out=sb_row[1:2, :], in_=brow[:])
                s_bc = sb.tile([128, HID], F32, tag="s_bc")
                nc.sync.dma_start(out=s_bc[:],
                                  in_=sb_row[0:1, :].partition_broadcast(
                                      128).squeeze(1))
                b_bc = sb.tile([128, HID], F32, tag="b_bc")
                nc.sync.dma_start(out=b_bc[:],
                                  in_=sb_row[1:2, :].partition_broadcast(
                                      128).squeeze(1))

                # ---- node phase: BN + elu + residual (+ next tables) ----
                for t in range(ntile):
                    hg = sb.tile([128, 128], F32, tag="hg_n")
                    nc.sync.dma_start(out=hg[:],
                                      in_=hagg_d[t * 128:(t + 1) * 128, :])
                    rs = sb.tile([128, 128], F32, tag="rs_n")
                    nc.sync.dma_start(out=rs[:],
                                      in_=res_cur[t * 128:(t + 1) * 128, :])
                    u = sb.tile([128, 128], F32, tag="u_n")
                    nc.vector.tensor_tensor(out=u[:], in0=hg[:], in1=s_bc[:],
                                            op=OP.mult)
                    nc.vector.tensor_tensor(out=u[:], in0=u[:], in1=b_bc[:],
                                            op=OP.add)
                    r = sb.tile([128, 128], F32, tag="r_n")
                    nc.vector.tensor_scalar_max(out=r[:], in0=u[:],
                                                scalar1=0.0)
                    mn = sb.tile([128, 128], F32, tag="mn_n")
                    nc.vector.tensor_scalar_min(out=mn[:], in0=u[:],
                                                scalar1=0.0)
                    em = sb.tile([128, 128], F32, tag="em_n")
                    nc.scalar.activation(em[:], mn[:], AF.Exp)
                    hn = sb.tile([128, 128], F32, tag="hn_n")
                    nc.vector.tensor_tensor(out=hn[:], in0=r[:], in1=em[:],
                                            op=OP.add)
                    nc.vector.tensor_scalar_add(out=hn[:], in0=hn[:],
                                                scalar1=-1.0)
                    nc.vector.tensor_tensor(out=hn[:], in0=hn[:], in1=rs[:],
                                            op=OP.add)
                    if li < L - 1:
                        nc.sync.dma_start(
                            out=res_nxt[t * 128:(t + 1) * 128, :], in_=hn[:])
                    tp = pp.tile([128, 128], F32, space="PSUM", tag="mm_ps")
                    nc.tensor.transpose(tp[:], in_=hn[:], identity=ident[:])
                    hT = sb.tile([128, 128], F32, tag="hT_n")
                    nc.vector.tensor_copy(out=hT[:], in_=tp[:])
                    if li < L - 1:
                        write_ext(hT, li + 1, t)
                    else:
                        fps = pp.tile([128, 1], F32, space="PSUM",
                                      tag="mm_ps")
                        nc.tensor.matmul(fps[:], lhsT=hT[:], rhs=fcw_sb[:],
                                         start=True, stop=True)
                        ov = sb.tile([128, 1], F32, tag="ov")
                        nc.scalar.activation(ov[:], fps[:], AF.Identity,
                                             bias=fcb_sb[:, 0:1])
                        nc.sync.dma_start(
                            out=out.ap()[t * 128:(t + 1) * 128, :],
                            in_=ov[:])
                res_cur, res_nxt = res_nxt, res_cur

    nc.compile()
    return nc


def _run(x, edge_index, proj_w, proj_b, W, att_src, att_dst, conv_b,
         bn_g, bn_b, fc_w, fc_b, n_nodes, shard, trace=False):
    x = np.asarray(x, np.float32)
    edge_index = np.asarray(edge_index, np.int64)
    cfg = host_prep(x, edge_index, n_nodes, shard)
    nc = build_program(cfg, n_nodes, shard)

    # fold attention vectors into the layer weight: B = W @ A, A[(h,c),h']=a
    wext_np = np.zeros((L, 128, ROWB), np.float32)
    for li in range(L):
        A_s = np.zeros((HID, HEADS), np.float32)
        A_d = np.zeros((HID, HEADS), np.float32)
        for h in range(HEADS):
            A_s[h * CPH:(h + 1) * CPH, h] = att_src[li, h]
            A_d[h * CPH:(h + 1) * CPH, h] = att_dst[li, h]
        wext_np[li, :, :128] = W[li]
        wext_np[li, :, 128:136] = W[li] @ A_s
        wext_np[li, :, 136:144] = W[li] @ A_d

    common = {
        "pw": np.ascontiguousarray(
            np.asarray(proj_w, np.float32).reshape(2, 128, HID)),
        "pb": np.asarray(proj_b, np.float32).reshape(128, 1),
        "wext": wext_np,
        "bng": np.asarray(bn_g, np.float32).reshape(L, 1, HID),
        "bnb": np.asarray(bn_b, np.float32).reshape(L, 1, HID),
        "fcw": np.asarray(fc_w, np.float32).reshape(128, 1),
        "fcb": np.full((128, 1), np.asarray(fc_b).reshape(-1)[0], np.float32),
    }
    in_maps = []
    for c in range(NCORES):
        m = dict(common)
        m["xt"] = cfg["xts"][c]
        m["sg"] = cfg["sg16"][c]
        m["dl"] = cfg["dl16"][c]
        m["dlb"] = cfg["dlb"][c]
        in_maps.append(m)

    res = run_bass_kernel_spmd(nc, in_maps, core_ids=list(range(NCORES)),
                               trace=trace)
    outs = [res.results[c]["out"][:min(shard, n_nodes - c * shard)]
            for c in range(NCORES)]
    full = np.concatenate(outs, axis=0).astype(np.float32)
    return full, res


def kernel(x, edge_index, proj_w, proj_b, W, att_src, att_dst, conv_b,
           bn_g, bn_b, fc_w, fc_b):
    full, _ = _run(x, edge_index, proj_w, proj_b, W, att_src, att_dst,
                   conv_b, bn_g, bn_b, fc_w, fc_b,
                   n_nodes=100000, shard=12500)
    return full
